# revision 1
# baseline (speedup 1.0000x reference)
"""Trainium2 Bass kernel for nn_MoEConnectionProcessor.

Data-parallel over cells: 8 cores x 2560 padded cells (19683 real).
Per core the cell range is processed in 40 "superblocks" of 64 cells
(= 13 subtiles of 128 edges, since 64*26 = 13*128 = 1664).

Layout strategy:
  - neighbor data loaded twice in bf16: natural [edge, d] tiles (for
    masked-aggregation matmuls, contract over edges) and DMA-transposed
    [d, edge] tiles (stationary operand for the per-edge message
    projection, giving natural-layout messages in PSUM).
  - all "second stage" activations live transposed [d, cell]; biases
    become per-partition ACT bias vectors there.
  - masked sums (mask = conn_type compare) are PE matmuls whose moving
    operand is a per-edge-scaled staircase matrix built in bulk on DVE.
  - 1/count normalization is applied at aggregate-evacuation time via a
    PE one-hot broadcast (bf16 hi+lo for fp32-grade accuracy).
"""

import numpy as np
import ml_dtypes
from contextlib import ExitStack

N_CELLS, K, D, HG = 19683, 26, 128, 64
NCORES = 8
NS = 2560                 # padded cells per core
SBC = 64                  # cells per superblock
NSB = NS // SBC           # 40 superblocks
NSUB = 13                 # subtiles (128 edges) per superblock
EPB = NSUB * 128          # 1664 edges per superblock
E = NS * K                # 66560 edges per core
NSUBT = NS * K // 128     # 520 subtiles per core
CHUNK = 512
NCHUNK = NS // CHUNK      # 5
SB_PER_CHUNK = CHUNK // SBC  # 8
CNF_STEPS, DTC = 3, 0.1

bf16 = ml_dtypes.bfloat16


def _staircase_consts():
    """Per-class (subtile position within superblock) staircase matrices."""
    # cb[chi]: first local cell of subtile chi; j = cell_local - cb in [0, 6)
    S6 = np.zeros((13, 128, 6), np.float32)
    S64T = np.zeros((13, 64, 128), np.float32)
    cbs = []
    for chi in range(13):
        cb = (chi * 128) // K
        cbs.append(cb)
        for p in range(128):
            cl = (chi * 128 + p) // K     # local cell 0..63
            S6[chi, p, cl - cb] = 1.0
            S64T[chi, cl, p] = 1.0
    return S6, S64T, cbs


S6_CLS, S64T_CLS, CB_LOC = _staircase_consts()


def _consts():
    c = {}
    # S6_big [128, 520*6], S12_big [128, 520*12] tiled over all subtiles
    s6 = np.tile(S6_CLS.transpose(1, 0, 2).reshape(128, 13 * 6), (1, NSB))
    # order must be (s_global, j): s_global = t*13 + chi -> col s*6 + j
    s6 = np.concatenate([S6_CLS[s % 13] for s in range(NSUBT)], axis=1)
    c["S6_big"] = s6.astype(bf16)                       # [128, 3120]
    s12 = np.concatenate(
        [np.repeat(S6_CLS[s % 13], 2, axis=1) for s in range(NSUBT)], axis=1)
    c["S12_big"] = s12.astype(bf16)                     # [128, 6240]
    s64 = np.concatenate([S64T_CLS[chi] for chi in range(13)], axis=1)
    c["S64T_all"] = np.concatenate([s64, s64], axis=0).astype(bf16)  # [128,1664]
    oh = np.zeros((3, 3 * 128), np.float32)
    for m in range(3):
        oh[m, m * 128:(m + 1) * 128] = 1.0
    c["OH3"] = oh.astype(bf16)                          # [3, 384]
    ident = np.eye(128, dtype=np.float32)
    c["IDENT"] = ident.astype(bf16)                     # [128, 128]
    c["ONES3"] = np.ones((3, 1), np.float32).astype(bf16)
    return c


CONSTS = _consts()


def _enable_ldw_opt():
    # compile_bir_kernel hardcodes --enable-ldw-opt=false; rewrite it so
    # walrus can optimize LDWEIGHTS scheduling for this bf16-only kernel.
    from concourse import bass_utils as bu
    if getattr(bu, "_ldw_patched", False):
        return
    orig = bu.run_command

    def run_command(cmd, *a, **k):
        cmd = [c.replace("--enable-ldw-opt=false", "--enable-ldw-opt=false")
               if isinstance(c, str) else c for c in cmd]
        return orig(cmd, *a, **k)

    bu.run_command = run_command
    bu._ldw_patched = True
    try:
        from concourse import bass2jax as b2j
        if getattr(b2j, "run_command", None) is orig:
            b2j.run_command = run_command
    except Exception:
        pass


def _build_bass():
    import concourse.bass as bass
    import concourse.tile as tile
    from concourse import bacc, mybir

    _enable_ldw_opt()

    f32, bft, i32 = mybir.dt.float32, mybir.dt.bfloat16, mybir.dt.int32
    AF = mybir.ActivationFunctionType
    OP = mybir.AluOpType
    AX = mybir.AxisListType

    nc = bacc.Bacc("TRN2", target_bir_lowering=False, debug=False,
                   num_devices=NCORES)

    def din(name, shape, dt):
        return nc.dram_tensor(name, shape, dt, kind="ExternalInput").ap()

    nbr = din("nbr", [E, D], bft)
    nbr_nat = din("nbr_nat", [128, NSUBT * D], bft)
    curT_f = din("curT_f", [D, NS], f32)
    curT_b = din("curT_b", [D, NS], bft)
    conn = din("conn", [128, NSUBT], i32)
    wnames = ["Wl1", "Wl2", "Wm1", "Wm2", "Wu1", "Wu2", "Wc1", "Wc2"]
    W = {k: din(k, [D, D], bft) for k in wnames}
    W["Wg1"] = din("Wg1", [D, HG], bft)
    W["Wg2"] = din("Wg2", [HG, 3], bft)
    bias_in = {
        "b_local": din("b_local", [D, 1], f32),
        "b_upd": din("b_upd", [D, 1], f32),
        "b_cnf": din("b_cnf", [D, 1], f32),
        "b_msg": din("b_msg", [D, 1], f32),
        "b_g1": din("b_g1", [HG, 1], f32),
        "b_g2": din("b_g2", [3, 1], f32),
    }
    S6_d = din("S6_big", [128, NSUBT * 6], bft)
    S12_d = din("S12_big", [128, NSUBT * 12], bft)
    S64T_d = din("S64T_all", [128, 13 * 128], bft)
    OH3_d = din("OH3", [3, 384], bft)
    ID_d = din("IDENT", [128, 128], bft)
    ONES3_d = din("ONES3", [3, 1], bft)
    outT = nc.dram_tensor("outT", [D, NS], f32, kind="ExternalOutput").ap()

    with tile.TileContext(nc) as tc, ExitStack() as ctx:
        const = ctx.enter_context(tc.tile_pool(name="const", bufs=1))
        build = ctx.enter_context(tc.tile_pool(name="build", bufs=1))
        big = ctx.enter_context(tc.tile_pool(name="big", bufs=1))
        stream = ctx.enter_context(tc.tile_pool(name="stream", bufs=2))
        temp1 = ctx.enter_context(tc.tile_pool(name="temp1", bufs=1))
        ps_long = ctx.enter_context(tc.tile_pool(name="ps_long", bufs=2,
                                                 space="PSUM"))
        ps = ctx.enter_context(tc.tile_pool(name="ps", bufs=4, space="PSUM"))

        # ---------- load constants / weights ----------
        wt = {}
        for k in wnames:
            t = const.tile([D, D], bft, tag=k)
            nc.sync.dma_start(t[:], W[k][:])
            wt[k] = t
        wg1 = const.tile([D, HG], bft)
        nc.sync.dma_start(wg1[:], W["Wg1"][:])
        wg2 = const.tile([HG, 3], bft)
        nc.sync.dma_start(wg2[:], W["Wg2"][:])
        bias = {}
        for k, ap in bias_in.items():
            t = const.tile(list(ap.shape), mybir.dt.float32, tag=k)
            nc.sync.dma_start(t[:], ap[:])
            bias[k] = t
        s6c = build.tile([128, NSUBT * 6], bft)
        nc.sync.dma_start(s6c[:], S6_d[:])
        s12c = build.tile([128, NSUBT * 12], bft)
        nc.sync.dma_start(s12c[:], S12_d[:])
        s64t = const.tile([128, 13 * 128], bft)
        nc.sync.dma_start(s64t[:], S64T_d[:])
        oh3 = const.tile([3, 384], bft)
        nc.sync.dma_start(oh3[:], OH3_d[:])
        ident = const.tile([128, 128], bft)
        nc.sync.dma_start(ident[:], ID_d[:])
        ones3 = const.tile([3, 1], bft)
        nc.sync.dma_start(ones3[:], ONES3_d[:])
        curTb = const.tile([D, NS], bft)
        nc.sync.dma_start(curTb[:], curT_b[:])
        curTf = const.tile([D, NS], mybir.dt.float32)
        nc.sync.dma_start(curTf[:], curT_f[:])
        conn_sb = const.tile([128, NSUBT], i32)
        nc.sync.dma_start(conn_sb[:], conn[:])
        zrow = const.tile([1, 128], bft)
        nc.vector.memset(zrow[:], 0.0)
        zdum = const.tile([1, CHUNK], bft)
        nc.vector.memset(zdum[:], 0.0)

        # ---------- bulk mask building ----------
        w3 = big.tile([128, NSUBT * 3], bft)         # col s*3+m, m in (l,d,f)
        for m, val in enumerate((0, 2, 1)):          # l->conn==0 d->2 f->1
            nc.vector.tensor_scalar(w3[:, m::3], conn_sb[:], val, None,
                                    OP.is_equal)
        B_ld = big.tile([128, NSUBT * 12], bft)
        w3v = w3[:].rearrange("p (s c) -> p s c", c=3)
        in1 = w3v[:, :, 0:2].unsqueeze(2).broadcast_to([128, NSUBT, 6, 2])
        nc.vector.tensor_tensor(
            B_ld[:].rearrange("p (s j c) -> p s j c", j=6, c=2),
            s12c[:].rearrange("p (s j c) -> p s j c", j=6, c=2),
            in1, OP.mult)
        B_f = big.tile([128, NSUBT * 6], bft)
        in1f = w3v[:, :, 2:3].broadcast_to([128, NSUBT, 6])
        nc.vector.tensor_tensor(
            B_f[:].rearrange("p (s j) -> p s j", j=6),
            s6c[:].rearrange("p (s j) -> p s j", j=6),
            in1f, OP.mult)

        # ---------- cpmT = Wm1.T @ curT + b_msg ;  cpm_nat per superblock ----
        cpmT = big.tile([D, NS], bft)
        for ch in range(NCHUNK):
            pm = ps.tile([128, CHUNK], mybir.dt.float32, tag="p")
            sl = slice(ch * CHUNK, (ch + 1) * CHUNK)
            nc.tensor.matmul(pm[:], wt["Wm1"][:], curTb[:, sl], start=True,
                             stop=True)
            nc.scalar.activation(cpmT[:, sl], pm[:], AF.Identity,
                                 bias=bias["b_msg"][:])
        cpm_nat = big.tile([128, NSB * 128], bft)
        for t in range(NSB):
            pt = ps.tile([64, 128], bft, tag="p")
            nc.tensor.transpose(pt[:], cpmT[:, t * 64:(t + 1) * 64], ident[:])
            nc.scalar.copy(cpm_nat[0:64, t * 128:(t + 1) * 128], pt[:])
            nc.scalar.copy(cpm_nat[64:128, t * 128:(t + 1) * 128], pt[:])

        # ---------- main superblock loop ----------
        aggldT = big.tile([128, NSB * 128], bft)   # col t*128 + 2c+m
        aggfT = big.tile([128, NSB * 64], bft)     # col t*64 + c
        def do_superblock(t):
            natT = stream.tile([128, EPB], bft, tag="natT")
            nc.sync.dma_start(natT[:], nbr[t * EPB:(t + 1) * EPB, :],
                              transpose=True)
            nat = stream.tile([128, NSUB, 128], bft, tag="nat")
            nc.sync.dma_start(
                nat[:], nbr_nat[:, t * EPB:(t + 1) * EPB].rearrange(
                    "p (s d) -> p s d", d=128))
            msgs = stream.tile([128, EPB], bft, tag="msgs")

            pagg = ps_long.tile([128, 192], mybir.dt.float32, tag="pagg")
            nc.vector.memset(pagg[:], 0.0)

            groups = [(0, 4), (4, 4), (8, 4), (12, 1)]
            for g0, gn in groups:
                pmsg = ps.tile([128, 512], mybir.dt.float32, tag="p")
                for i in range(gn):
                    s = g0 + i
                    sg = t * NSUB + s
                    csl = slice(i * 128, (i + 1) * 128)
                    nc.tensor.matmul(pmsg[:, csl],
                                     natT[:, s * 128:(s + 1) * 128],
                                     wt["Wm2"][:], start=True, stop=False)
                    half = 64 * (s % 2)
                    nc.tensor.matmul(pmsg[:, csl],
                                     s64t[half:half + 64,
                                          (s % 13) * 128:(s % 13 + 1) * 128],
                                     cpm_nat[half:half + 64,
                                             t * 128:(t + 1) * 128],
                                     start=False, stop=True)
                nc.scalar.activation(msgs[:, g0 * 128:(g0 + gn) * 128],
                                     pmsg[:, 0:gn * 128], AF.Relu)
                for i in range(gn):
                    s = g0 + i
                    sg = t * NSUB + s
                    cb2 = 2 * CB_LOC[s]
                    w = min(6, SBC - CB_LOC[s])
                    last = s == NSUB - 1
                    nc.tensor.matmul(pagg[:, cb2:cb2 + 2 * w],
                                     nat[:, s, :],
                                     B_ld[:, sg * 12:sg * 12 + 2 * w],
                                     start=False, stop=last)
                    nc.tensor.matmul(pagg[:, 128 + CB_LOC[s]:128 + CB_LOC[s] + w],
                                     msgs[:, s * 128:(s + 1) * 128],
                                     B_f[:, sg * 6:sg * 6 + w],
                                     start=False, stop=last)
            return pagg

        def evac_superblock(t, pagg):
            # evacuate aggregates with 1/cnt scaling (cell-indexed cols)
            csl = slice(t * SBC, (t + 1) * SBC)
            nc.vector.tensor_tensor(aggldT[:, t * 128:(t + 1) * 128:2],
                                    pagg[:, 0:128:2], ibc[0][:, csl], OP.mult)
            nc.vector.tensor_tensor(aggldT[:, t * 128 + 1:(t + 1) * 128:2],
                                    pagg[:, 1:128:2], ibc[1][:, csl], OP.mult)
            nc.vector.tensor_tensor(aggfT[:, t * 64:(t + 1) * 64],
                                    pagg[:, 128:192], ibc[2][:, csl], OP.mult)

        early = [do_superblock(t) for t in range(2)]

        # ---------- counts -> inv (cell layout [3, NS]) ----------
        inv_hi = big.tile([3, NS], bft)
        inv_lo = big.tile([3, NS], bft)
        for ch in range(NCHUNK):
            pc = ps.tile([3, CHUNK], mybir.dt.float32, tag="p")
            nc.vector.memset(pc[:], 0.0)
            s0 = ch * SB_PER_CHUNK * NSUB
            for sl in range(SB_PER_CHUNK * NSUB):
                s = s0 + sl
                cb = (s // NSUB) * SBC - ch * CHUNK + CB_LOC[s % NSUB]
                w = min(6, SBC - CB_LOC[s % NSUB])
                last = sl == SB_PER_CHUNK * NSUB - 1
                nc.tensor.matmul(pc[:, cb:cb + w], w3[:, 3 * s:3 * s + 3],
                                 s6c[:, 6 * s:6 * s + w], start=False,
                                 stop=last)
            csl = slice(ch * CHUNK, (ch + 1) * CHUNK)
            cnt1 = temp1.tile([3, CHUNK], mybir.dt.float32, tag="cnt1")
            nc.vector.tensor_scalar(cnt1[:], pc[:], 1.0, None, OP.max)
            invf = temp1.tile([3, CHUNK], mybir.dt.float32, tag="invf")
            nc.vector.reciprocal(invf[:], cnt1[:])
            nc.vector.tensor_copy(inv_hi[:, csl], invf[:])
            lo_t = temp1.tile([3, CHUNK], mybir.dt.float32, tag="lot")
            nc.vector.tensor_tensor(lo_t[:], invf[:], inv_hi[:, csl],
                                    OP.subtract)
            nc.vector.tensor_copy(inv_lo[:, csl], lo_t[:])

        # broadcast inv rows to 128 partitions (bf16, hi+lo): ibc[m]
        ibc = []
        for m in range(3):
            t = big.tile([128, NS], bft, tag=f"ibc{m}")
            ibc.append(t)
        for m in range(3):
            for ch in range(NCHUNK):
                pb = ps.tile([128, CHUNK], mybir.dt.float32, tag="p")
                sl = slice(ch * CHUNK, (ch + 1) * CHUNK)
                nc.tensor.matmul(pb[:], oh3[:, m * 128:(m + 1) * 128],
                                 inv_hi[:, sl], start=True, stop=False)
                mm = nc.tensor.matmul(pb[:], oh3[:, m * 128:(m + 1) * 128],
                                       inv_lo[:, sl], start=False, stop=True)
                mm.ins.ldweights = False
                nc.scalar.copy(ibc[m][:, sl], pb[:])


        for t, pg in enumerate(early):
            evac_superblock(t, pg)
        for t in range(2, NSB):
            evac_superblock(t, do_superblock(t))

        # ---------- second stage (transposed, chunked) ----------
        localT = big.tile([128, NS], bft)
        funcT = big.tile([128, NS], bft)

        def agg_view(base_off, ch):
            # aggldT cols (t*128 + 2c + m) for cells of chunk ch
            v = aggldT[:, ch * SB_PER_CHUNK * 128 + base_off:
                       (ch + 1) * SB_PER_CHUNK * 128:2]
            return v.rearrange("p (t c) -> p t c", c=64)

        for ch in range(NCHUNK):
            sl = slice(ch * CHUNK, (ch + 1) * CHUNK)
            pl = ps.tile([128, CHUNK], mybir.dt.float32, tag="p")
            nc.tensor.matmul(pl[:], wt["Wl1"][:], curTb[:, sl], start=True,
                             stop=False)
            nc.tensor.matmul(
                pl[:].rearrange("p (t c) -> p t c", c=64),
                wt["Wl2"][:], agg_view(0, ch), start=False, stop=True)
            nc.scalar.activation(localT[:, sl], pl[:], AF.Tanh,
                                 bias=bias["b_local"][:])
            pf = ps.tile([128, CHUNK], mybir.dt.float32, tag="p")
            nc.tensor.matmul(pf[:], wt["Wu1"][:], curTb[:, sl], start=True,
                             stop=False)
            nc.tensor.matmul(
                pf[:].rearrange("p (t c) -> p t c", c=64),
                wt["Wu2"][:],
                aggfT[:, ch * SB_PER_CHUNK * 64:(ch + 1) * SB_PER_CHUNK * 64]
                .rearrange("p (t c) -> p t c", c=64),
                start=False, stop=True)
            nc.scalar.activation(funcT[:, sl], pf[:], AF.Tanh,
                                 bias=bias["b_upd"][:])

        # CNF: 3 Euler steps
        s_prev = curTf
        s_prev_bf = curTb
        for step in range(CNF_STEPS):
            s_next = big.tile([128, NS], mybir.dt.float32, tag=f"s{step % 2}")
            for ch in range(NCHUNK):
                sl = slice(ch * CHUNK, (ch + 1) * CHUNK)
                pp = ps.tile([128, CHUNK], mybir.dt.float32, tag="p")
                nc.tensor.matmul(pp[:], wt["Wc1"][:], s_prev_bf[:, sl],
                                 start=True, stop=False)
                nc.tensor.matmul(
                    pp[:].rearrange("p (t c) -> p t c", c=64),
                    wt["Wc2"][:], agg_view(1, ch), start=False, stop=True)
                th = temp1.tile([128, CHUNK], mybir.dt.float32, tag="th")
                nc.scalar.activation(th[:], pp[:], AF.Tanh,
                                     bias=bias["b_cnf"][:])
                nc.vector.tensor_scalar(th[:], th[:], DTC, None, OP.mult)
                nc.vector.tensor_tensor(s_next[:, sl], s_prev[:, sl], th[:],
                                        OP.add)
            s_prev = s_next
            if step < CNF_STEPS - 1:
                nb = big.tile([128, NS], bft, tag="sbf")
                nc.vector.tensor_copy(nb[:], s_next[:])
                s_prev_bf = nb

        # gating + final mix, per chunk
        for ch in range(NCHUNK):
            sl = slice(ch * CHUNK, (ch + 1) * CHUNK)
            ph = ps.tile([HG, CHUNK], mybir.dt.float32, tag="p")
            nc.tensor.matmul(ph[:], wg1[:], curTb[:, sl], start=True,
                             stop=True)
            hT = temp1.tile([HG, CHUNK], bft, tag="hT")
            nc.scalar.activation(hT[:], ph[:], AF.Relu, bias=bias["b_g1"][:])
            pz = ps.tile([3, CHUNK], mybir.dt.float32, tag="p")
            nc.tensor.matmul(pz[:], wg2[:], hT[:], start=True, stop=True)
            e3 = temp1.tile([3, CHUNK], mybir.dt.float32, tag="e3")
            nc.scalar.activation(e3[:], pz[:], AF.Exp, bias=bias["b_g2"][:])
            e_hi = temp1.tile([3, CHUNK], bft, tag="ehi")
            nc.vector.tensor_copy(e_hi[:], e3[:])
            e_lof = temp1.tile([3, CHUNK], mybir.dt.float32, tag="elof")
            nc.vector.tensor_tensor(e_lof[:], e3[:], e_hi[:], OP.subtract)
            e_lo = temp1.tile([3, CHUNK], bft, tag="elo")
            nc.vector.tensor_copy(e_lo[:], e_lof[:])
            psum1 = ps.tile([1, CHUNK], mybir.dt.float32, tag="p")
            nc.tensor.matmul(psum1[:], ones3[:], e_hi[:], start=True,
                             stop=False)
            mm = nc.tensor.matmul(psum1[:], ones3[:], e_lo[:], start=False,
                                  stop=True)
            mm.ins.ldweights = False
            rec = temp1.tile([1, CHUNK], mybir.dt.float32, tag="rec")
            nc.vector.reciprocal(rec[:], psum1[:])
            rbc = temp1.tile([128, CHUNK], mybir.dt.float32, tag="rbc")
            nc.gpsimd.partition_broadcast(rbc[:], rec[:])

            pe = []
            for m in range(3):
                p = ps.tile([128, CHUNK], mybir.dt.float32, tag="p")
                nc.tensor.matmul(p[:], oh3[:, m * 128:(m + 1) * 128],
                                 e_hi[:], start=True, stop=False)
                mm = nc.tensor.matmul(p[:], oh3[:, m * 128:(m + 1) * 128],
                                       e_lo[:], start=False, stop=True)
                mm.ins.ldweights = False
                pe.append(p)
            acc = temp1.tile([128, CHUNK], mybir.dt.float32, tag="acc")
            tmp = temp1.tile([128, CHUNK], mybir.dt.float32, tag="tmp")
            nc.vector.tensor_tensor(acc[:], localT[:, sl], pe[0][:], OP.mult)
            nc.vector.tensor_tensor(tmp[:], funcT[:, sl], pe[1][:], OP.mult)
            nc.vector.tensor_tensor(acc[:], acc[:], tmp[:], OP.add)
            nc.vector.tensor_tensor(tmp[:], s_prev[:, sl], pe[2][:], OP.mult)
            nc.vector.tensor_tensor(acc[:], acc[:], tmp[:], OP.add)
            nc.vector.tensor_tensor(acc[:], acc[:], rbc[:], OP.mult)
            nc.sync.dma_start(outT[:, sl], acc[:])

    nc.compile()
    return nc


_NC_CACHE = None


def _get_nc():
    global _NC_CACHE
    if _NC_CACHE is None:
        _NC_CACHE = _build_bass()
    return _NC_CACHE


def _prep_core_inputs(cur, nbr, conn, weights):
    """cur [NS, D] f32, nbr [NS, K, D] f32, conn [NS, K] i32 -> input map."""
    m = {}
    nf = nbr.reshape(E, D).astype(bf16)
    m["nbr"] = nf
    m["nbr_nat"] = np.ascontiguousarray(
        nf.reshape(NSUBT, 128, D).transpose(1, 0, 2)).reshape(128, NSUBT * D)
    ct = np.ascontiguousarray(cur.T)
    m["curT_f"] = ct.astype(np.float32)
    m["curT_b"] = ct.astype(bf16)
    m["conn"] = np.ascontiguousarray(
        conn.reshape(NSUBT, 128).T).astype(np.int32)
    Wl, Wm, Wu, Wc = (weights["W_local"], weights["W_msg"],
                      weights["W_upd"], weights["W_cnf"])
    m["Wl1"], m["Wl2"] = Wl[:D].astype(bf16), Wl[D:].astype(bf16)
    m["Wm1"], m["Wm2"] = Wm[:D].astype(bf16), Wm[D:].astype(bf16)
    m["Wu1"], m["Wu2"] = Wu[:D].astype(bf16), Wu[D:].astype(bf16)
    m["Wc1"], m["Wc2"] = Wc[:D].astype(bf16), Wc[D:].astype(bf16)
    m["Wg1"] = weights["W_g1"].astype(bf16)
    m["Wg2"] = weights["W_g2"].astype(bf16)
    m["b_local"] = weights["b_local"].reshape(D, 1).astype(np.float32)
    m["b_upd"] = weights["b_upd"].reshape(D, 1).astype(np.float32)
    m["b_cnf"] = weights["b_cnf"].reshape(D, 1).astype(np.float32)
    m["b_msg"] = weights["b_msg"].reshape(D, 1).astype(np.float32)
    m["b_g1"] = weights["b_g1"].reshape(HG, 1).astype(np.float32)
    m["b_g2"] = weights["b_g2"].reshape(3, 1).astype(np.float32)
    for k, v in CONSTS.items():
        m[k] = v
    return m


def kernel(**inputs):
    from concourse.bass_utils import run_bass_kernel_spmd

    cur = np.asarray(inputs["current_state"], np.float32)
    nbr = np.asarray(inputs["neighbor_states"], np.float32)
    conn = np.asarray(inputs["conn_type"], np.int32)
    weights = {k: np.asarray(v, np.float32) for k, v in inputs.items()
               if k not in ("current_state", "neighbor_states", "conn_type")}

    npad = NCORES * NS
    cur_p = np.zeros((npad, D), np.float32)
    cur_p[:N_CELLS] = cur
    nbr_p = np.zeros((npad, K, D), np.float32)
    nbr_p[:N_CELLS] = nbr
    conn_p = np.full((npad, K), 3, np.int32)
    conn_p[:N_CELLS] = conn

    in_maps = []
    for c in range(NCORES):
        sl = slice(c * NS, (c + 1) * NS)
        in_maps.append(_prep_core_inputs(cur_p[sl], nbr_p[sl], conn_p[sl],
                                         weights))
    nc = _get_nc()
    res = run_bass_kernel_spmd(nc, in_maps, list(range(NCORES)))
    out = np.concatenate([res.results[c]["outT"].T for c in range(NCORES)],
                         axis=0)
    return np.ascontiguousarray(out[:N_CELLS]).astype(np.float32)


if __name__ == "__main__":
    pass



# revision 3
# speedup vs baseline: 2.1513x; 2.1513x over previous
"""Trainium2 Bass kernel for nn_MoEConnectionProcessor.

Data-parallel over cells: 8 cores x 2560 padded cells (19683 real).
Per core the cell range is processed in 40 superblocks of 64 cells
(= 13 subtiles of 128 edges, since 64*26 = 13*128 = 1664).

v2 design notes:
  - all mask/count/scale math is folded host-side into the staircase
    aggregation constants (B_ld, B_f), so the device does no counts,
    reciprocals or normalization for the aggregates.
  - neighbor data ships in two host-prepared layouts (edge-major for
    aggregation stationaries, d-major for the projection stationary);
    no on-device DMA transpose.
  - the per-edge cur-projection (q) term is added by a staircase matmul
    whose K dimension carries an extra all-ones row: row 64 of the
    moving operand holds b_msg, folding the bias in for free.
  - msgs relu/evacuation runs on DVE (tensor_scalar max), not ACT.
  - CNF is reparametrized as u = s/DT: each Euler step is one f32 add;
    1/DT is folded into the distant staircase scales and DT into the
    ACT tanh input scale and the distant gate broadcast constant.
  - gating softmax uses reciprocal_approx_fast + PE row-broadcasts;
    stage 2 is emitted per 512-cell chunk as soon as that chunk's
    aggregates complete, so it overlaps the remaining superblocks.
"""

import numpy as np
import ml_dtypes
from contextlib import ExitStack

N_CELLS, K, D, HG = 19683, 26, 128, 64
NCORES = 8
NS = 2560                 # padded cells per core
SBC = 64                  # cells per superblock
NSB = NS // SBC           # 40 superblocks
NSUB = 13                 # subtiles (128 edges) per superblock
EPB = NSUB * 128          # 1664 edges per superblock
E = NS * K                # 66560 edges per core
NSUBT = NS * K // 128     # 520 subtiles per core
CHUNK = 512
NCHUNK = NS // CHUNK      # 5
SB_PER_CHUNK = CHUNK // SBC  # 8
CNF_STEPS, DTC = 3, 0.1

bf16 = ml_dtypes.bfloat16

# first local cell of each subtile class
CB_LOC = [(chi * 128) // K for chi in range(NSUB)]


def _consts():
    c = {}
    # S65 [65, 13*128]: row c_local one-hot per edge column, row 64 ones
    s65 = np.zeros((65, NSUB * 128), np.float32)
    idx = np.arange(NSUB * 128)
    s65[idx // K, idx] = 1.0
    s65[64, :] = 1.0
    c["S65"] = s65.astype(bf16)
    # one-hot blocks for gate broadcast; distant block carries DT
    oh = np.zeros((3, 3 * 128), np.float32)
    for m in range(3):
        oh[m, m * 128:(m + 1) * 128] = 1.0 if m < 2 else DTC
    c["OH3"] = oh.astype(bf16)
    c["ONES3"] = np.ones((3, 1), np.float32).astype(bf16)
    c["ONES13"] = np.ones((1, 3), np.float32).astype(bf16)
    return c


CONSTS = _consts()


def _build_bass():
    import concourse.bass as bass
    import concourse.tile as tile
    from concourse import bacc, mybir

    f32, bft, i32 = mybir.dt.float32, mybir.dt.bfloat16, mybir.dt.int32
    AF = mybir.ActivationFunctionType
    OP = mybir.AluOpType

    nc = bacc.Bacc("TRN2", target_bir_lowering=False, debug=False,
                   num_devices=NCORES)

    def din(name, shape, dt):
        return nc.dram_tensor(name, shape, dt, kind="ExternalInput").ap()

    nbr_nat = din("nbr_nat", [128, NSUBT * D], bft)   # edge-major subtiles
    natT = din("natT", [128, E], bft)                 # d-major (pretransposed)
    curT_b = din("curT_b", [D, NS], bft)
    u0f_d = din("u0f", [D, NS], f32)                  # cur.T / DT
    u0b_d = din("u0b", [D, NS], bft)
    Bld_d = din("B_ld", [128, NSUBT * 12], bft)       # scaled l/d staircases
    Bf_d = din("B_f", [128, NSUBT * 6], bft)          # scaled f staircase
    bmsg_tiled = din("bmsg_tiled", [1, NSB * 128], bft)
    wnames = ["Wl1", "Wl2", "Wm1", "Wm2", "Wu1", "Wu2", "Wc1", "Wc2"]
    W = {k: din(k, [D, D], bft) for k in wnames}
    W["Wg1"] = din("Wg1", [D, HG], bft)
    W["Wg2"] = din("Wg2", [HG, 3], bft)
    bias_in = {
        "b_local": din("b_local", [D, 1], f32),
        "b_upd": din("b_upd", [D, 1], f32),
        "b_cnf": din("b_cnf", [D, 1], f32),
        "b_g1": din("b_g1", [HG, 1], f32),
        "b_g2": din("b_g2", [3, 1], f32),
    }
    S65_d = din("S65", [65, NSUB * 128], bft)
    OH3_d = din("OH3", [3, 384], bft)
    ONES3_d = din("ONES3", [3, 1], bft)
    ONES13_d = din("ONES13", [1, 3], bft)
    outT = nc.dram_tensor("outT", [D, NS], f32, kind="ExternalOutput").ap()

    with tile.TileContext(nc) as tc, ExitStack() as ctx:
        const = ctx.enter_context(tc.tile_pool(name="const", bufs=1))
        big = ctx.enter_context(tc.tile_pool(name="big", bufs=1))
        nat_p = ctx.enter_context(tc.tile_pool(name="nat", bufs=3))
        natT_p = ctx.enter_context(tc.tile_pool(name="natT", bufs=3))
        msgs_p = ctx.enter_context(tc.tile_pool(name="msgs", bufs=2))
        s2tmp = ctx.enter_context(tc.tile_pool(name="s2tmp", bufs=2))
        ps_proj = ctx.enter_context(tc.tile_pool(name="ps_proj", bufs=2,
                                                 space="PSUM"))
        ps_agg = ctx.enter_context(tc.tile_pool(name="ps_agg", bufs=2,
                                                space="PSUM"))
        ps_cpm = ctx.enter_context(tc.tile_pool(name="ps_cpm", bufs=1,
                                                space="PSUM"))
        ps_s2 = ctx.enter_context(tc.tile_pool(name="ps_s2", bufs=3,
                                               space="PSUM"))

        # ---------- load constants / weights ----------
        wt = {}
        for k in wnames:
            t = const.tile([D, D], bft, tag=k)
            nc.sync.dma_start(t[:], W[k][:])
            wt[k] = t
        wg1 = const.tile([D, HG], bft)
        nc.sync.dma_start(wg1[:], W["Wg1"][:])
        wg2 = const.tile([HG, 3], bft)
        nc.sync.dma_start(wg2[:], W["Wg2"][:])
        bias = {}
        for k, ap in bias_in.items():
            t = const.tile(list(ap.shape), f32, tag=k)
            nc.sync.dma_start(t[:], ap[:])
            bias[k] = t
        s65 = const.tile([65, NSUB * 128], bft)
        nc.sync.dma_start(s65[:], S65_d[:])
        oh3 = const.tile([3, 384], bft)
        nc.sync.dma_start(oh3[:], OH3_d[:])
        ones3 = const.tile([3, 1], bft)
        nc.sync.dma_start(ones3[:], ONES3_d[:])
        ones13 = const.tile([1, 3], bft)
        nc.sync.dma_start(ones13[:], ONES13_d[:])
        curTb = const.tile([D, NS], bft)
        nc.sync.dma_start(curTb[:], curT_b[:])
        u0f = const.tile([D, NS], f32)
        nc.sync.dma_start(u0f[:], u0f_d[:])
        u0b = const.tile([D, NS], bft)
        nc.sync.dma_start(u0b[:], u0b_d[:])
        bld = const.tile([128, NSUBT * 12], bft)
        nc.sync.dma_start(bld[:], Bld_d[:])
        bfc = const.tile([128, NSUBT * 6], bft)
        nc.sync.dma_start(bfc[:], Bf_d[:])

        # persistent activation buffers
        aggldT = big.tile([128, NSB * 128], bft)   # col t*128 + 2c+m
        aggfT = big.tile([128, NSB * 64], bft)     # col t*64 + c
        cpm = big.tile([65, NSB * 128], bft)       # rows 0-63 cur@Wm1, 64 bias
        nc.sync.dma_start(cpm[64:65, :], bmsg_tiled[:])
        localT = big.tile([128, NS], bft)
        funcT = big.tile([128, NS], bft)
        hT = big.tile([HG, NS], bft)
        e3 = big.tile([3, NS], bft)
        recf = big.tile([1, NS], f32)
        recb = big.tile([1, NS], bft)
        gates = big.tile([3, NS], bft)
        u_f = [big.tile([128, NS], f32, name=f"uf{i}", tag=f"uf{i}")
               for i in range(2)]
        u_b = big.tile([128, NS], bft)

        # ---------- gating (only needs curTb; runs while nat DMAs land) ----
        for ch in range(NCHUNK):
            sl = slice(ch * CHUNK, (ch + 1) * CHUNK)
            ph = ps_s2.tile([HG, CHUNK], f32, tag="p")
            nc.tensor.matmul(ph[:], wg1[:], curTb[:, sl], start=True,
                             stop=True)
            nc.scalar.activation(hT[:, sl], ph[:], AF.Relu, bias=bias["b_g1"])
        for ch in range(NCHUNK):
            sl = slice(ch * CHUNK, (ch + 1) * CHUNK)
            pz = ps_s2.tile([3, CHUNK], f32, tag="p")
            nc.tensor.matmul(pz[:], wg2[:], hT[:, sl], start=True, stop=True)
            nc.scalar.activation(e3[:, sl], pz[:], AF.Exp, bias=bias["b_g2"])
        for ch in range(NCHUNK):
            sl = slice(ch * CHUNK, (ch + 1) * CHUNK)
            pgs = ps_s2.tile([1, CHUNK], f32, tag="p")
            nc.tensor.matmul(pgs[:], ones3[:], e3[:, sl], start=True,
                             stop=True)
            nc.vector.reciprocal_approx_fast(recf[:, sl], pgs[:])
        nc.vector.tensor_copy(recb[:], recf[:])
        for ch in range(NCHUNK):
            sl = slice(ch * CHUNK, (ch + 1) * CHUNK)
            pr3 = ps_s2.tile([3, CHUNK], f32, tag="p")
            nc.tensor.matmul(pr3[:], ones13[:], recb[:, sl], start=True,
                             stop=True)
            nc.vector.tensor_tensor(gates[:, sl], e3[:, sl], pr3[:], OP.mult)

        # ---------- main superblock loop ----------
        def cpm_mm(t):
            pc = ps_cpm.tile([SBC, 128], f32, tag="pc")
            nc.tensor.matmul(pc[:], curTb[:, t * SBC:(t + 1) * SBC],
                             wt["Wm1"][:], start=True, stop=True)
            nc.vector.tensor_copy(cpm[0:SBC, t * 128:(t + 1) * 128], pc[:])

        def do_superblock(t):
            natT_t = natT_p.tile([128, EPB], bft, tag="natT")
            nc.sync.dma_start(natT_t[:], natT[:, t * EPB:(t + 1) * EPB])
            nat_t = nat_p.tile([128, NSUB, 128], bft, tag="nat")
            nc.sync.dma_start(
                nat_t[:], nbr_nat[:, t * EPB:(t + 1) * EPB].rearrange(
                    "p (s d) -> p s d", d=128))
            msgs = msgs_p.tile([128, EPB], bft, tag="msgs")
            pagg = ps_agg.tile([128, 192], f32, tag="pagg")
            nc.vector.memset(pagg[:], 0.0)

            groups = [(0, 4), (4, 4), (8, 4), (12, 1)]
            for g0, gn in groups:
                pmsg = ps_proj.tile([128, 512], f32, tag="p")
                for i in range(gn):
                    s = g0 + i
                    csl = slice(i * 128, (i + 1) * 128)
                    nc.tensor.matmul(pmsg[:, csl],
                                     natT_t[:, s * 128:(s + 1) * 128],
                                     wt["Wm2"][:], start=True, stop=False)
                    nc.tensor.matmul(pmsg[:, csl],
                                     s65[:, s * 128:(s + 1) * 128],
                                     cpm[:, t * 128:(t + 1) * 128],
                                     start=False, stop=True)
                nc.vector.tensor_scalar(msgs[:, g0 * 128:(g0 + gn) * 128],
                                        pmsg[:, 0:gn * 128], 0.0, None,
                                        OP.max)
                for i in range(gn):
                    s = g0 + i
                    sg = t * NSUB + s
                    cb = CB_LOC[s]
                    w = min(6, SBC - cb)
                    last = s == NSUB - 1
                    nc.tensor.matmul(pagg[:, 2 * cb:2 * cb + 2 * w],
                                     nat_t[:, s, :],
                                     bld[:, sg * 12:sg * 12 + 2 * w],
                                     start=False, stop=last)
                    nc.tensor.matmul(pagg[:, 128 + cb:128 + cb + w],
                                     msgs[:, s * 128:(s + 1) * 128],
                                     bfc[:, sg * 6:sg * 6 + w],
                                     start=False, stop=last)
            return pagg

        def evac_superblock(t, pagg):
            nc.vector.tensor_copy(aggldT[:, t * 128:(t + 1) * 128],
                                  pagg[:, 0:128])
            nc.vector.tensor_copy(aggfT[:, t * 64:(t + 1) * 64],
                                  pagg[:, 128:192])

        # ---------- stage 2, per 512-cell chunk ----------
        def agg_view(base_off, ch):
            v = aggldT[:, ch * SB_PER_CHUNK * 128 + base_off:
                       (ch + 1) * SB_PER_CHUNK * 128:2]
            return v.rearrange("p (t c) -> p t c", c=64)

        def stage2_chunk(ch):
            sl = slice(ch * CHUNK, (ch + 1) * CHUNK)
            pl = ps_s2.tile([128, CHUNK], f32, tag="p")
            nc.tensor.matmul(pl[:], wt["Wl1"][:], curTb[:, sl], start=True,
                             stop=False)
            nc.tensor.matmul(
                pl[:].rearrange("p (t c) -> p t c", c=64),
                wt["Wl2"][:], agg_view(0, ch), start=False, stop=True)
            nc.scalar.activation(localT[:, sl], pl[:], AF.Tanh,
                                 bias=bias["b_local"])
            pf = ps_s2.tile([128, CHUNK], f32, tag="p")
            nc.tensor.matmul(pf[:], wt["Wu1"][:], curTb[:, sl], start=True,
                             stop=False)
            nc.tensor.matmul(
                pf[:].rearrange("p (t c) -> p t c", c=64),
                wt["Wu2"][:],
                aggfT[:, ch * SB_PER_CHUNK * 64:(ch + 1) * SB_PER_CHUNK * 64]
                .rearrange("p (t c) -> p t c", c=64),
                start=False, stop=True)
            nc.scalar.activation(funcT[:, sl], pf[:], AF.Tanh,
                                 bias=bias["b_upd"])

            ub_prev, uf_prev = u0b, u0f
            for step in range(CNF_STEPS):
                pp = ps_s2.tile([128, CHUNK], f32, tag="p")
                nc.tensor.matmul(pp[:], wt["Wc1"][:], ub_prev[:, sl],
                                 start=True, stop=False)
                nc.tensor.matmul(
                    pp[:].rearrange("p (t c) -> p t c", c=64),
                    wt["Wc2"][:], agg_view(1, ch), start=False, stop=True)
                th = s2tmp.tile([128, CHUNK], f32, tag="th")
                nc.scalar.activation(th[:], pp[:], AF.Tanh,
                                     bias=bias["b_cnf"], scale=DTC)
                uf_next = u_f[step % 2]
                nc.vector.tensor_tensor(uf_next[:, sl], uf_prev[:, sl],
                                        th[:], OP.add)
                if step < CNF_STEPS - 1:
                    nc.vector.tensor_copy(u_b[:, sl], uf_next[:, sl])
                    ub_prev = u_b
                uf_prev = uf_next

            pe = []
            for m in range(3):
                p = ps_s2.tile([128, CHUNK], f32, tag="p")
                nc.tensor.matmul(p[:], oh3[:, m * 128:(m + 1) * 128],
                                 gates[:, sl], start=True, stop=True)
                pe.append(p)
            acc = s2tmp.tile([128, CHUNK], f32, tag="acc")
            tmp = s2tmp.tile([128, CHUNK], f32, tag="tmp")
            nc.vector.tensor_tensor(acc[:], localT[:, sl], pe[0][:], OP.mult)
            nc.vector.tensor_tensor(tmp[:], funcT[:, sl], pe[1][:], OP.mult)
            nc.vector.tensor_tensor(acc[:], acc[:], tmp[:], OP.add)
            nc.vector.tensor_tensor(tmp[:], uf_prev[:, sl], pe[2][:], OP.mult)
            nc.vector.tensor_tensor(acc[:], acc[:], tmp[:], OP.add)
            nc.sync.dma_start(outT[:, sl], acc[:])

        cpm_mm(0)
        cpm_mm(1)
        for t in range(NSB):
            if t + 2 < NSB:
                cpm_mm(t + 2)
            evac_superblock(t, do_superblock(t))
            if (t + 1) % SB_PER_CHUNK == 0:
                stage2_chunk((t + 1) // SB_PER_CHUNK - 1)

    nc.compile()
    return nc


_NC_CACHE = None


def _get_nc():
    global _NC_CACHE
    if _NC_CACHE is None:
        _NC_CACHE = _build_bass()
    return _NC_CACHE


def _prep_core_inputs(cur, nbr, conn, weights):
    """cur [NS, D] f32, nbr [NS, K, D] f32, conn [NS, K] i32 -> input map."""
    m = {}
    nf = nbr.reshape(E, D).astype(bf16)
    m["nbr_nat"] = np.ascontiguousarray(
        nf.reshape(NSUBT, 128, D).transpose(1, 0, 2)).reshape(128, NSUBT * D)
    m["natT"] = np.ascontiguousarray(nf.T)
    ct = np.ascontiguousarray(cur.T)
    m["curT_b"] = ct.astype(bf16)
    u0 = ct.astype(np.float32) / DTC
    m["u0f"] = u0
    m["u0b"] = u0.astype(bf16)

    # host-side masks, counts and scales folded into staircases
    ctype = conn.reshape(E)
    ml = ctype == 0
    mf = ctype == 1
    md = ctype == 2
    cnt = lambda mm: np.maximum(mm.reshape(NS, K).sum(1).astype(np.float32),
                                1.0)
    cl, cf, cd = cnt(ml), cnt(mf), cnt(md)
    e = np.arange(E)
    s = e // 128
    p = e % 128
    c = e // K
    j = c - (s * 128) // K
    wl_e = ml / cl[c]
    wd_e = md / (cd[c] * DTC)
    wf_e = mf / cf[c]
    B_ld = np.zeros((128, NSUBT * 12), np.float32)
    B_ld[p, s * 12 + 2 * j] = wl_e
    B_ld[p, s * 12 + 2 * j + 1] = wd_e
    B_f = np.zeros((128, NSUBT * 6), np.float32)
    B_f[p, s * 6 + j] = wf_e
    m["B_ld"] = B_ld.astype(bf16)
    m["B_f"] = B_f.astype(bf16)

    Wl, Wm, Wu, Wc = (weights["W_local"], weights["W_msg"],
                      weights["W_upd"], weights["W_cnf"])
    m["Wl1"], m["Wl2"] = Wl[:D].astype(bf16), Wl[D:].astype(bf16)
    m["Wm1"], m["Wm2"] = Wm[:D].astype(bf16), Wm[D:].astype(bf16)
    m["Wu1"], m["Wu2"] = Wu[:D].astype(bf16), Wu[D:].astype(bf16)
    m["Wc1"], m["Wc2"] = Wc[:D].astype(bf16), Wc[D:].astype(bf16)
    m["Wg1"] = weights["W_g1"].astype(bf16)
    m["Wg2"] = weights["W_g2"].astype(bf16)
    m["b_local"] = weights["b_local"].reshape(D, 1).astype(np.float32)
    m["b_upd"] = weights["b_upd"].reshape(D, 1).astype(np.float32)
    m["b_cnf"] = weights["b_cnf"].reshape(D, 1).astype(np.float32)
    m["b_g1"] = weights["b_g1"].reshape(HG, 1).astype(np.float32)
    m["b_g2"] = weights["b_g2"].reshape(3, 1).astype(np.float32)
    m["bmsg_tiled"] = np.tile(
        weights["b_msg"].reshape(1, D), (1, NSB)).astype(bf16)
    for k, v in CONSTS.items():
        m[k] = v
    return m


def kernel(**inputs):
    from concourse.bass_utils import run_bass_kernel_spmd

    cur = np.asarray(inputs["current_state"], np.float32)
    nbr = np.asarray(inputs["neighbor_states"], np.float32)
    conn = np.asarray(inputs["conn_type"], np.int32)
    weights = {k: np.asarray(v, np.float32) for k, v in inputs.items()
               if k not in ("current_state", "neighbor_states", "conn_type")}

    npad = NCORES * NS
    cur_p = np.zeros((npad, D), np.float32)
    cur_p[:N_CELLS] = cur
    nbr_p = np.zeros((npad, K, D), np.float32)
    nbr_p[:N_CELLS] = nbr
    conn_p = np.full((npad, K), 3, np.int32)
    conn_p[:N_CELLS] = conn

    in_maps = []
    for c in range(NCORES):
        sl = slice(c * NS, (c + 1) * NS)
        in_maps.append(_prep_core_inputs(cur_p[sl], nbr_p[sl], conn_p[sl],
                                         weights))
    nc = _get_nc()
    res = run_bass_kernel_spmd(nc, in_maps, list(range(NCORES)))
    out = np.concatenate([res.results[c]["outT"].T for c in range(NCORES)],
                         axis=0)
    return np.ascontiguousarray(out[:N_CELLS]).astype(np.float32)


if __name__ == "__main__":
    pass


# revision 13
# speedup vs baseline: 2.5592x; 1.1896x over previous
"""Trainium2 Bass kernel for nn_MoEConnectionProcessor.

Data-parallel over cells: 8 cores x 2560 padded cells (19683 real).
Per core the cell range is processed in 40 superblocks of 64 cells
(= 13 subtiles of 128 edges, since 64*26 = 13*128 = 1664).

v2 design notes:
  - all mask/count/scale math is folded host-side into the staircase
    aggregation constants (B_ld, B_f), so the device does no counts,
    reciprocals or normalization for the aggregates.
  - neighbor data ships in two host-prepared layouts (edge-major for
    aggregation stationaries, d-major for the projection stationary);
    no on-device DMA transpose.
  - the per-edge cur-projection (q) term is added by a staircase matmul
    whose K dimension carries an extra all-ones row: row 64 of the
    moving operand holds b_msg, folding the bias in for free.
  - msgs relu/evacuation runs on DVE (tensor_scalar max), not ACT.
  - CNF is reparametrized as u = s/DT: each Euler step is one f32 add;
    1/DT is folded into the distant staircase scales and DT into the
    ACT tanh input scale and the distant gate broadcast constant.
  - gating softmax uses reciprocal_approx_fast + PE row-broadcasts;
    stage 2 is emitted per 512-cell chunk as soon as that chunk's
    aggregates complete, so it overlaps the remaining superblocks.
"""

import numpy as np
import ml_dtypes
from contextlib import ExitStack

N_CELLS, K, D, HG = 19683, 26, 128, 64
NCORES = 8
NS = 2560                 # padded cells per core
SBC = 64                  # cells per superblock
NSB = NS // SBC           # 40 superblocks
NSUB = 13                 # subtiles (128 edges) per superblock
EPB = NSUB * 128          # 1664 edges per superblock
E = NS * K                # 66560 edges per core
NSUBT = NS * K // 128     # 520 subtiles per core
CHUNK = 512
NCHUNK = NS // CHUNK      # 5
SB_PER_CHUNK = CHUNK // SBC  # 8
CNF_STEPS, DTC = 3, 0.1

bf16 = ml_dtypes.bfloat16

# first local cell of each subtile class
CB_LOC = [(chi * 128) // K for chi in range(NSUB)]


def _consts():
    c = {}
    # S65 [65, 13*128]: row c_local one-hot per edge column, row 64 ones
    s65 = np.zeros((65, NSUB * 128), np.float32)
    idx = np.arange(NSUB * 128)
    s65[idx // K, idx] = 1.0
    s65[64, :] = 1.0
    c["S65"] = s65.astype(bf16)
    # one-hot blocks for gate broadcast; distant block carries DT
    oh = np.zeros((3, 3 * 128), np.float32)
    for m in range(3):
        oh[m, m * 128:(m + 1) * 128] = 1.0 if m < 2 else DTC
    c["OH3"] = oh.astype(bf16)
    c["ONES3"] = np.ones((3, 1), np.float32).astype(bf16)
    c["ONES13"] = np.ones((1, 3), np.float32).astype(bf16)
    return c


CONSTS = _consts()


def _build_bass():
    import concourse.bass as bass
    import concourse.tile as tile
    from concourse import bacc, mybir

    f32, bft, i32 = mybir.dt.float32, mybir.dt.bfloat16, mybir.dt.int32
    AF = mybir.ActivationFunctionType
    OP = mybir.AluOpType

    nc = bacc.Bacc("TRN2", target_bir_lowering=False, debug=False,
                   num_devices=NCORES)

    def din(name, shape, dt):
        return nc.dram_tensor(name, shape, dt, kind="ExternalInput").ap()

    nbr_nat = din("nbr_nat", [128, NSUBT * D], bft)   # edge-major subtiles
    natT = din("natT", [128, E], bft)                 # d-major (pretransposed)
    curT_b = din("curT_b", [D, NS], bft)
    u0f_d = din("u0f", [D, NS], f32)                  # cur.T / DT
    u0b_d = din("u0b", [D, NS], bft)
    Bld_d = din("B_ld", [128, NSUBT * 12], bft)       # scaled l/d staircases
    Bf_d = din("B_f", [128, NSUBT * 6], bft)          # scaled f staircase
    bmsg_tiled = din("bmsg_tiled", [1, NSB * 128], bft)
    wnames = ["Wl1", "Wl2", "Wm1", "Wm2", "Wu1", "Wu2", "Wc1", "Wc2"]
    W = {k: din(k, [D, D], bft) for k in wnames}
    W["Wg1"] = din("Wg1", [D, HG], bft)
    W["Wg2"] = din("Wg2", [HG, 3], bft)
    bias_in = {
        "b_local": din("b_local", [D, 1], f32),
        "b_upd": din("b_upd", [D, 1], f32),
        "b_cnf": din("b_cnf", [D, 1], f32),
        "b_g1": din("b_g1", [HG, 1], f32),
        "b_g2": din("b_g2", [3, 1], f32),
    }
    S65_d = din("S65", [65, NSUB * 128], bft)
    OH3_d = din("OH3", [3, 384], bft)
    ONES3_d = din("ONES3", [3, 1], bft)
    ONES13_d = din("ONES13", [1, 3], bft)
    outT = nc.dram_tensor("outT", [D, NS], f32, kind="ExternalOutput").ap()

    with tile.TileContext(nc) as tc, ExitStack() as ctx:
        const = ctx.enter_context(tc.tile_pool(name="const", bufs=1))
        big = ctx.enter_context(tc.tile_pool(name="big", bufs=1))
        nat_p = ctx.enter_context(tc.tile_pool(name="nat", bufs=4))
        natT_p = ctx.enter_context(tc.tile_pool(name="natT", bufs=4))
        msgs_p = ctx.enter_context(tc.tile_pool(name="msgs", bufs=4))
        s2tmp = ctx.enter_context(tc.tile_pool(name="s2tmp", bufs=2))
        ps_proj = ctx.enter_context(tc.tile_pool(name="ps_proj", bufs=3,
                                                 space="PSUM"))
        ps_agg = ctx.enter_context(tc.tile_pool(name="ps_agg", bufs=2,
                                                space="PSUM"))
        ps_cpm = ctx.enter_context(tc.tile_pool(name="ps_cpm", bufs=1,
                                                space="PSUM"))
        ps_s2 = ctx.enter_context(tc.tile_pool(name="ps_s2", bufs=2,
                                               space="PSUM"))

        # ---------- load constants / weights ----------
        wt = {}
        for k in wnames:
            t = const.tile([D, D], bft, tag=k)
            nc.sync.dma_start(t[:], W[k][:])
            wt[k] = t
        wg1 = const.tile([D, HG], bft)
        nc.sync.dma_start(wg1[:], W["Wg1"][:])
        wg2 = const.tile([HG, 3], bft)
        nc.sync.dma_start(wg2[:], W["Wg2"][:])
        bias = {}
        for k, ap in bias_in.items():
            t = const.tile(list(ap.shape), f32, tag=k)
            nc.sync.dma_start(t[:], ap[:])
            bias[k] = t
        s65 = const.tile([65, NSUB * 128], bft)
        nc.sync.dma_start(s65[:], S65_d[:])
        oh3 = const.tile([3, 384], bft)
        nc.sync.dma_start(oh3[:], OH3_d[:])
        ones3 = const.tile([3, 1], bft)
        nc.sync.dma_start(ones3[:], ONES3_d[:])
        ones13 = const.tile([1, 3], bft)
        nc.sync.dma_start(ones13[:], ONES13_d[:])
        curTb = const.tile([D, NS], bft)
        nc.sync.dma_start(curTb[:], curT_b[:])
        u0f = const.tile([D, NS], f32)
        nc.sync.dma_start(u0f[:], u0f_d[:])
        u0b = const.tile([D, NS], bft)
        nc.sync.dma_start(u0b[:], u0b_d[:])
        bld = const.tile([128, NSUBT * 12], bft)
        nc.sync.dma_start(bld[:], Bld_d[:])
        bfc = const.tile([128, NSUBT * 6], bft)
        nc.sync.dma_start(bfc[:], Bf_d[:])

        # persistent activation buffers
        aggldT = big.tile([128, NSB * 128], bft)   # col t*128 + 2c+m
        aggfT = big.tile([128, NSB * 64], bft)     # col t*64 + c
        cpm = big.tile([65, NSB * 128], bft)       # rows 0-63 cur@Wm1, 64 bias
        nc.sync.dma_start(cpm[64:65, :], bmsg_tiled[:])
        localT = big.tile([128, NS], bft)
        funcT = big.tile([128, NS], bft)
        hT = big.tile([HG, NS], bft)
        e3 = big.tile([3, NS], bft)
        recf = big.tile([1, NS], f32)
        recb = big.tile([1, NS], bft)
        gates = big.tile([3, NS], bft)
        u_f = [big.tile([128, NS], f32, name=f"uf{i}", tag=f"uf{i}")
               for i in range(2)]
        u_b = big.tile([128, NS], bft)

        # ---------- gating (only needs curTb; runs while nat DMAs land) ----
        for ch in range(NCHUNK):
            sl = slice(ch * CHUNK, (ch + 1) * CHUNK)
            ph = ps_s2.tile([HG, CHUNK], f32, tag="p")
            nc.tensor.matmul(ph[:], wg1[:], curTb[:, sl], start=True,
                             stop=True)
            nc.scalar.activation(hT[:, sl], ph[:], AF.Relu, bias=bias["b_g1"])
        for ch in range(NCHUNK):
            sl = slice(ch * CHUNK, (ch + 1) * CHUNK)
            pz = ps_s2.tile([3, CHUNK], f32, tag="p")
            nc.tensor.matmul(pz[:], wg2[:], hT[:, sl], start=True, stop=True)
            nc.scalar.activation(e3[:, sl], pz[:], AF.Exp, bias=bias["b_g2"])
        for ch in range(NCHUNK):
            sl = slice(ch * CHUNK, (ch + 1) * CHUNK)
            pgs = ps_s2.tile([1, CHUNK], f32, tag="p")
            nc.tensor.matmul(pgs[:], ones3[:], e3[:, sl], start=True,
                             stop=True)
            nc.vector.reciprocal_approx_fast(recf[:, sl], pgs[:])
        nc.vector.tensor_copy(recb[:], recf[:])
        for ch in range(NCHUNK):
            sl = slice(ch * CHUNK, (ch + 1) * CHUNK)
            pr3 = ps_s2.tile([3, CHUNK], f32, tag="p")
            nc.tensor.matmul(pr3[:], ones13[:], recb[:, sl], start=True,
                             stop=True)
            nc.vector.tensor_tensor(gates[:, sl], e3[:, sl], pr3[:], OP.mult)

        # ---------- main superblock loop ----------
        def cpm_mm(t2):
            pc = ps_cpm.tile([SBC, 256], f32, tag="pc")
            for h in range(2):
                t = t2 + h
                nc.tensor.matmul(pc[:, h * 128:(h + 1) * 128],
                                 curTb[:, t * SBC:(t + 1) * SBC],
                                 wt["Wm1"][:], start=True, stop=True)
            nc.vector.tensor_copy(cpm[0:SBC, t2 * 128:(t2 + 2) * 128], pc[:])

        # Superblocks are processed in pairs (t, t+1) as a flat stream of
        # "pair-groups" of up to 4 subtile classes.  The S65 staircase
        # stationary is loaded once per class and reused for the second
        # superblock via ldweights=False.  Aggregation matmuls of pair-group
        # k are emitted after the projections of pair-group k+1 so the PE
        # never waits on the relu evacuation (which alternates DVE/ACT).
        GROUPS = [(0, 4), (4, 4), (8, 4), (12, 1)]

        class PairState:
            pass

        def pair_open(t):
            st = PairState()
            st.t = t
            st.natT, st.nat, st.msgs, st.pagg = [], [], [], []
            for u in (t, t + 1):
                natT_t = natT_p.tile([128, EPB], bft, tag="natT",
                                     name=f"natT{u}")
                nc.sync.dma_start(natT_t[:], natT[:, u * EPB:(u + 1) * EPB])
                st.natT.append(natT_t)
                nat_t = nat_p.tile([128, NSUB, 128], bft, tag="nat",
                                   name=f"nat{u}")
                nc.sync.dma_start(
                    nat_t[:], nbr_nat[:, u * EPB:(u + 1) * EPB].rearrange(
                        "p (s d) -> p s d", d=128))
                st.nat.append(nat_t)
                st.msgs.append(msgs_p.tile([128, EPB], bft, tag="msgs",
                                           name=f"msgs{u}"))
            st.pagg2 = ps_agg.tile([128, 384], f32, tag="pagg",
                                    name=f"pagg{t}")
            st.pagg = [st.pagg2[:, 0:192], st.pagg2[:, 192:384]]
            return st

        def pair_projq(st, gi):
            g0, gn = GROUPS[gi]
            st_pm = []
            for h in range(2):
                st_pm.append(ps_proj.tile([128, 512], f32, tag="p",
                                          name=f"pm{h}"))
            for i in range(gn):
                s = g0 + i
                csl = slice(i * 128, (i + 1) * 128)
                for h in range(2):
                    nc.tensor.matmul(st_pm[h][:, csl],
                                     st.natT[h][:, s * 128:(s + 1) * 128],
                                     wt["Wm2"][:], start=True, stop=False)
                for h in range(2):
                    mm = nc.tensor.matmul(
                        st_pm[h][:, csl], s65[:, s * 128:(s + 1) * 128],
                        cpm[:, (st.t + h) * 128:(st.t + h + 1) * 128],
                        start=False, stop=True)
                    if h == 1:
                        mm.ins.ldweights = False
            # relu evacuation: one half on DVE, the other on ACT
            sl = slice(g0 * 128, (g0 + gn) * 128)
            nc.vector.tensor_scalar(st.msgs[0][:, sl], st_pm[0][:, 0:gn * 128],
                                    0.0, None, OP.max)
            nc.scalar.activation(st.msgs[1][:, sl], st_pm[1][:, 0:gn * 128],
                                 AF.Relu)

        def pair_agg(st, gi):
            if gi == 0:
                nc.vector.memset(st.pagg2[:], 0.0)
            g0, gn = GROUPS[gi]
            for i in range(gn):
                s = g0 + i
                cb = CB_LOC[s]
                w = min(6, SBC - cb)
                last = s == NSUB - 1
                for h in range(2):
                    sg = (st.t + h) * NSUB + s
                    nc.tensor.matmul(st.pagg[h][:, 2 * cb:2 * cb + 2 * w],
                                     st.nat[h][:, s, :],
                                     bld[:, sg * 12:sg * 12 + 2 * w],
                                     start=False, stop=last)
                    nc.tensor.matmul(st.pagg[h][:, 128 + cb:128 + cb + w],
                                     st.msgs[h][:, s * 128:(s + 1) * 128],
                                     bfc[:, sg * 6:sg * 6 + w],
                                     start=False, stop=last)

        def pair_evac(st):
            for h in range(2):
                u = st.t + h
                nc.vector.tensor_copy(aggldT[:, u * 128:(u + 1) * 128],
                                      st.pagg[h][:, 0:128])
                nc.vector.tensor_copy(aggfT[:, u * 64:(u + 1) * 64],
                                      st.pagg[h][:, 128:192])

        # ---------- stage 2, per 512-cell chunk ----------
        def agg_view(base_off, ch):
            v = aggldT[:, ch * SB_PER_CHUNK * 128 + base_off:
                       (ch + 1) * SB_PER_CHUNK * 128:2]
            return v.rearrange("p (t c) -> p t c", c=64)

        def stage2_chunk(ch):
            sl = slice(ch * CHUNK, (ch + 1) * CHUNK)
            pl = ps_s2.tile([128, CHUNK], f32, tag="p")
            nc.tensor.matmul(pl[:], wt["Wl1"][:], curTb[:, sl], start=True,
                             stop=False)
            nc.tensor.matmul(
                pl[:].rearrange("p (t c) -> p t c", c=64),
                wt["Wl2"][:], agg_view(0, ch), start=False, stop=True)
            nc.scalar.activation(localT[:, sl], pl[:], AF.Tanh,
                                 bias=bias["b_local"])
            pf = ps_s2.tile([128, CHUNK], f32, tag="p")
            nc.tensor.matmul(pf[:], wt["Wu1"][:], curTb[:, sl], start=True,
                             stop=False)
            nc.tensor.matmul(
                pf[:].rearrange("p (t c) -> p t c", c=64),
                wt["Wu2"][:],
                aggfT[:, ch * SB_PER_CHUNK * 64:(ch + 1) * SB_PER_CHUNK * 64]
                .rearrange("p (t c) -> p t c", c=64),
                start=False, stop=True)
            nc.scalar.activation(funcT[:, sl], pf[:], AF.Tanh,
                                 bias=bias["b_upd"])

            ub_prev, uf_prev = u0b, u0f
            for step in range(CNF_STEPS):
                pp = ps_s2.tile([128, CHUNK], f32, tag="p")
                nc.tensor.matmul(pp[:], wt["Wc1"][:], ub_prev[:, sl],
                                 start=True, stop=False)
                nc.tensor.matmul(
                    pp[:].rearrange("p (t c) -> p t c", c=64),
                    wt["Wc2"][:], agg_view(1, ch), start=False, stop=True)
                th = s2tmp.tile([128, CHUNK], f32, tag="th")
                nc.scalar.activation(th[:], pp[:], AF.Tanh,
                                     bias=bias["b_cnf"], scale=DTC)
                uf_next = u_f[step % 2]
                nc.vector.tensor_tensor(uf_next[:, sl], uf_prev[:, sl],
                                        th[:], OP.add)
                if step < CNF_STEPS - 1:
                    nc.vector.tensor_copy(u_b[:, sl], uf_next[:, sl])
                    ub_prev = u_b
                uf_prev = uf_next

            acc = s2tmp.tile([128, CHUNK], f32, tag="acc")
            tmp = s2tmp.tile([128, CHUNK], f32, tag="tmp")
            experts = [localT[:, sl], funcT[:, sl], uf_prev[:, sl]]
            for m in range(3):
                p = ps_s2.tile([128, CHUNK], f32, tag="p")
                nc.tensor.matmul(p[:], oh3[:, m * 128:(m + 1) * 128],
                                 gates[:, sl], start=True, stop=True)
                if m == 0:
                    nc.vector.tensor_tensor(acc[:], experts[m], p[:], OP.mult)
                else:
                    nc.vector.tensor_tensor(tmp[:], experts[m], p[:], OP.mult)
                    nc.vector.tensor_tensor(acc[:], acc[:], tmp[:], OP.add)
            nc.sync.dma_start(outT[:, sl], acc[:])

        # flat software-pipelined stream over pair-groups:
        # agg(k) is emitted right after projq(k+1); DMA prefetched 1 pair
        # ahead so the PE never waits on neighbor loads.
        cpm_mm(0)
        NG = len(GROUPS)
        pending = None          # (st, gi) whose agg is not yet emitted
        prev = None
        nxt = pair_open(0)
        for t2 in range(0, NSB, 2):
            st = nxt
            if t2 + 2 < NSB:
                cpm_mm(t2 + 2)
                nxt = pair_open(t2 + 2)
            for gi in range(NG):
                pair_projq(st, gi)
                if pending is not None:
                    pair_agg(*pending)
                pending = (st, gi)
                if gi == 0 and prev is not None:
                    pair_evac(prev)
                    if (t2 // 2) % 4 == 0:
                        stage2_chunk(t2 // SB_PER_CHUNK - 1)
            prev = st
        pair_agg(*pending)
        pair_evac(st)
        stage2_chunk(NCHUNK - 1)

    nc.compile()
    return nc


_NC_CACHE = None


def _get_nc():
    global _NC_CACHE
    if _NC_CACHE is None:
        _NC_CACHE = _build_bass()
    return _NC_CACHE


def _prep_core_inputs(cur, nbr, conn, weights):
    """cur [NS, D] f32, nbr [NS, K, D] f32, conn [NS, K] i32 -> input map."""
    m = {}
    nf = nbr.reshape(E, D).astype(bf16)
    m["nbr_nat"] = np.ascontiguousarray(
        nf.reshape(NSUBT, 128, D).transpose(1, 0, 2)).reshape(128, NSUBT * D)
    m["natT"] = np.ascontiguousarray(nf.T)
    ct = np.ascontiguousarray(cur.T)
    m["curT_b"] = ct.astype(bf16)
    u0 = ct.astype(np.float32) / DTC
    m["u0f"] = u0
    m["u0b"] = u0.astype(bf16)

    # host-side masks, counts and scales folded into staircases
    ctype = conn.reshape(E)
    ml = ctype == 0
    mf = ctype == 1
    md = ctype == 2
    cnt = lambda mm: np.maximum(mm.reshape(NS, K).sum(1).astype(np.float32),
                                1.0)
    cl, cf, cd = cnt(ml), cnt(mf), cnt(md)
    e = np.arange(E)
    s = e // 128
    p = e % 128
    c = e // K
    j = c - (s * 128) // K
    wl_e = ml / cl[c]
    wd_e = md / (cd[c] * DTC)
    wf_e = mf / cf[c]
    B_ld = np.zeros((128, NSUBT * 12), np.float32)
    B_ld[p, s * 12 + 2 * j] = wl_e
    B_ld[p, s * 12 + 2 * j + 1] = wd_e
    B_f = np.zeros((128, NSUBT * 6), np.float32)
    B_f[p, s * 6 + j] = wf_e
    m["B_ld"] = B_ld.astype(bf16)
    m["B_f"] = B_f.astype(bf16)

    Wl, Wm, Wu, Wc = (weights["W_local"], weights["W_msg"],
                      weights["W_upd"], weights["W_cnf"])
    m["Wl1"], m["Wl2"] = Wl[:D].astype(bf16), Wl[D:].astype(bf16)
    m["Wm1"], m["Wm2"] = Wm[:D].astype(bf16), Wm[D:].astype(bf16)
    m["Wu1"], m["Wu2"] = Wu[:D].astype(bf16), Wu[D:].astype(bf16)
    m["Wc1"], m["Wc2"] = Wc[:D].astype(bf16), Wc[D:].astype(bf16)
    m["Wg1"] = weights["W_g1"].astype(bf16)
    m["Wg2"] = weights["W_g2"].astype(bf16)
    m["b_local"] = weights["b_local"].reshape(D, 1).astype(np.float32)
    m["b_upd"] = weights["b_upd"].reshape(D, 1).astype(np.float32)
    m["b_cnf"] = weights["b_cnf"].reshape(D, 1).astype(np.float32)
    m["b_g1"] = weights["b_g1"].reshape(HG, 1).astype(np.float32)
    m["b_g2"] = weights["b_g2"].reshape(3, 1).astype(np.float32)
    m["bmsg_tiled"] = np.tile(
        weights["b_msg"].reshape(1, D), (1, NSB)).astype(bf16)
    for k, v in CONSTS.items():
        m[k] = v
    return m


def kernel(**inputs):
    from concourse.bass_utils import run_bass_kernel_spmd

    cur = np.asarray(inputs["current_state"], np.float32)
    nbr = np.asarray(inputs["neighbor_states"], np.float32)
    conn = np.asarray(inputs["conn_type"], np.int32)
    weights = {k: np.asarray(v, np.float32) for k, v in inputs.items()
               if k not in ("current_state", "neighbor_states", "conn_type")}

    npad = NCORES * NS
    cur_p = np.zeros((npad, D), np.float32)
    cur_p[:N_CELLS] = cur
    nbr_p = np.zeros((npad, K, D), np.float32)
    nbr_p[:N_CELLS] = nbr
    conn_p = np.full((npad, K), 3, np.int32)
    conn_p[:N_CELLS] = conn

    in_maps = []
    for c in range(NCORES):
        sl = slice(c * NS, (c + 1) * NS)
        in_maps.append(_prep_core_inputs(cur_p[sl], nbr_p[sl], conn_p[sl],
                                         weights))
    nc = _get_nc()
    res = run_bass_kernel_spmd(nc, in_maps, list(range(NCORES)))
    out = np.concatenate([res.results[c]["outT"].T for c in range(NCORES)],
                         axis=0)
    return np.ascontiguousarray(out[:N_CELLS]).astype(np.float32)


if __name__ == "__main__":
    pass
